# revision 24
# baseline (speedup 1.0000x reference)
"""MoE transformer layer on 8 Trainium2 NeuronCores (single SPMD launch).

Sharding:
  - Attention: token-sharded with balanced causal blocks. Core c owns 128-row
    query blocks c and 15-c; it computes q/k/v for its 256 tokens, ropes q/k,
    and all-gathers K (d-major rope-split layout) and V (token-major), bf16.
  - The causal loop is core-uniform: query block 0 scans kv slots 0..6 plus
    its local diagonal block, query block 1 scans slots 0..14 plus diagonal.
    Slots past a core's causal frontier get -480 added to their raw logits by
    a rank-1 accumulating matmul (host-supplied flag row), so exp(s/8) -> 0.
  - Router / shared expert / residual: token-sharded, fully local.
  - Experts: expert-parallel, 2 per core. The host builds the routing plan
    from the `indices` input: per-expert token lists (top-2 duplicates
    merged), score-slot selectors, and padded row-gather indices into the
    all-gathered x_ffn table.
  - Host unshard: place each core's 256 output rows, then scatter-add the
    expert partials (already score-scaled on device).

Matmul operands bf16, fp32 PSUM accumulation; softmax/rmsnorm/residual fp32.
"""
import sys

sys.path.insert(0, "/opt/trn_rl_repo")

import numpy as np
import ml_dtypes

import concourse.bass as bass
import concourse.bacc as bacc
import concourse.tile as tile
from concourse import mybir
from concourse.bass_utils import run_bass_kernel_spmd
from concourse.masks import make_identity

NCORES = 8
S, D = 2048, 1024
H, HD = 16, 64
TE, TOPK, ED = 16, 2, 512
DS = 2048
EPS = 1e-5
THETA = 10000.0
NBLK = 16
BLK = 128
TPC = 2 * BLK

F32 = mybir.dt.float32
BF16 = mybir.dt.bfloat16
I32 = mybir.dt.int32
AF = mybir.ActivationFunctionType
OP = mybir.AluOpType

CDT = BF16
CDT_NP = ml_dtypes.bfloat16
NEG = -480.0            # raw-logit mask; exp((x + NEG)/8) -> e^-60 ~ 0
NSLOT0, NSLOT1 = 7, 15  # fixed full-chunk slot counts per query block


def _cast(x):
    return np.ascontiguousarray(np.asarray(x).astype(CDT_NP))


def _f32(x):
    return np.ascontiguousarray(np.asarray(x, dtype=np.float32))


def _owner(b):
    return (b, 0) if b < NCORES else ((NBLK - 1) - b, 1)


def _core_tokens(c):
    b0, b1 = c, (NBLK - 1) - c
    return np.concatenate([np.arange(b0 * BLK, (b0 + 1) * BLK),
                           np.arange(b1 * BLK, (b1 + 1) * BLK)])


def _row_of_token(t):
    b = t // BLK
    o, i = _owner(b)
    return o * TPC + i * BLK + (t % BLK)


# --------------------------------------------------------------------------
# host-side preparation
# --------------------------------------------------------------------------

def build_host_data(x_input, indices, values, attn_w, attn_o_w, attn_norm_w,
                    ffn_norm_w, ffn_experts, main_keys, main_bias,
                    output_coeff, ffn_up_w, ffn_down_w, shared_norm_w):
    x_input = _f32(x_input).reshape(S, D)
    indices = np.asarray(indices).astype(np.int64)
    values = _f32(values)
    attn_w = _f32(attn_w)
    attn_o_w = _f32(attn_o_w)
    attn_norm_w = _f32(attn_norm_w)
    ffn_norm_w = _f32(ffn_norm_w)
    ffn_experts = _f32(ffn_experts)
    main_keys = _f32(main_keys)
    main_bias = _f32(main_bias)
    output_coeff = _f32(output_coeff).reshape(D)
    ffn_up_w = _f32(ffn_up_w)
    ffn_down_w = _f32(ffn_down_w)
    shared_norm_w = _f32(shared_norm_w)

    w_eff = attn_w * attn_norm_w[None, :]
    perm_half = np.concatenate([h * HD + np.arange(HD // 2) for h in range(H)])
    perm_qk = np.concatenate([perm_half, perm_half + HD // 2])
    perm_full = np.concatenate([perm_qk, D + perm_qk, 2 * D + np.arange(D)])
    attn_wT = _cast(w_eff[perm_full, :].T)            # [D, 3D]
    attn_o_wT = _cast(attn_o_w.T)                     # [D(in), D(out)]
    mkeys = _cast(main_keys * ffn_norm_w[:, None])    # [D, TE]
    upT = _cast((ffn_up_w * ffn_norm_w[None, :]).T)   # [D, 2*DS]
    downT = _cast(ffn_down_w.T)                       # [DS, D]
    w_shared_bc = _f32(np.tile(shared_norm_w[None, :], (BLK, 1)))

    w1 = [_cast(ffn_experts[0, e] * ffn_norm_w[:, None]) for e in range(TE)]
    w2 = [_cast(ffn_experts[1, e] * ffn_norm_w[:, None]) for e in range(TE)]
    w3T = [_cast((ffn_experts[2, e] * output_coeff[:, None]).T) for e in range(TE)]

    tok_lists, sels = [], []
    for e in range(TE):
        m0 = indices[:, 0] == e
        m1 = indices[:, 1] == e
        toks = np.nonzero(m0 | m1)[0]
        sel = np.stack([m0[toks], m1[toks]], axis=1).astype(np.float32)
        tok_lists.append(toks)
        sels.append(sel)
    n_max = max(len(t) for t in tok_lists)
    n_pad = max(BLK, ((n_max + BLK - 1) // BLK) * BLK)

    idx_pads, sel_pads = [], []
    for e in range(TE):
        n_e = len(tok_lists[e])
        idx = np.zeros((n_pad, 1), np.int32)
        idx[:n_e, 0] = [_row_of_token(t) for t in tok_lists[e]]
        sel = np.zeros((n_pad, TOPK), np.float32)
        sel[:n_e] = sels[e]
        idx_pads.append(idx)
        sel_pads.append(sel)

    inv = (1.0 / THETA) ** (np.arange(0, HD, 2, dtype=np.float64) / HD)

    def rope_tiles(pos):
        ang = np.outer(pos.astype(np.float64), inv)
        bc = np.cos(ang).T.astype(np.float32)
        bs = np.sin(ang).T.astype(np.float32)
        return (_cast(np.tile(np.tile(bc, (4, 1)), (1, 4))),
                _cast(np.tile(np.tile(bs, (4, 1)), (1, 4))))

    mask_add = _f32(np.where(np.tril(np.ones((BLK, BLK), bool)), 0.0, NEG))

    # permutation matrices: split rope layout -> natural head layout.
    # natural chunk jn holds heads (2jn, 2jn+1); source x1 rows live in split
    # chunk jn//2, x2 rows in chunk 4+jn//2, at rows (h%4)*32 + freq.
    pmats = np.zeros((4, BLK, BLK), np.float32)   # [A_even, B_even, A_odd, B_odd]
    for par in range(2):
        for rn in range(BLK):
            hh, dd = rn // HD, rn % HD
            hmod4 = 2 * par + hh
            if dd < HD // 2:
                pmats[2 * par + 0, hmod4 * 32 + dd, rn] = 1.0
            else:
                pmats[2 * par + 1, hmod4 * 32 + (dd - HD // 2), rn] = 1.0
    perm_mats = _cast(pmats.transpose(1, 0, 2).reshape(BLK, 4 * BLK))

    # router one-hot selectors: rsel[t, k*TE + e] = (indices[t,k] == e)
    def router_sel(toks):
        rs = np.zeros((TPC, TOPK * TE), np.float32)
        for k in range(TOPK):
            rs[np.arange(TPC), k * TE + indices[toks, k]] = 1.0
        return rs

    in_maps = []
    for c in range(NCORES):
        toks = _core_tokens(c)
        qb = [c, (NBLK - 1) - c]
        cos_t, sin_t = rope_tiles(toks)
        base_logit = values[toks] + main_bias[indices[toks]]

        nf0 = np.zeros((1, NSLOT0 * BLK), np.float32)
        nf1 = np.zeros((1, NSLOT1 * BLK), np.float32)
        nf0[0, qb[0] * BLK:] = NEG
        nf1[0, qb[1] * BLK:] = NEG

        e0, e1 = 2 * c, 2 * c + 1
        in_maps.append({
            "x_blk": x_input[toks],
            "attn_wT": attn_wT,
            "attn_o_wT": attn_o_wT,
            "mkeys": mkeys,
            "upT": upT,
            "downT": downT,
            "w_shared_bc": w_shared_bc,
            "cos_t": cos_t,
            "sin_t": sin_t,
            "mask_add": mask_add,
            "perm_mats": perm_mats,
            "negflag0": _cast(nf0),
            "negflag1": _cast(nf1),
            "base_logit": _f32(base_logit),
            "router_sel": router_sel(toks),
            "w1a": w1[e0], "w2a": w2[e0], "w3Ta": w3T[e0],
            "w1b": w1[e1], "w2b": w2[e1], "w3Tb": w3T[e1],
            "idx_a": idx_pads[e0], "sel_a": sel_pads[e0],
            "idx_b": idx_pads[e1], "sel_b": sel_pads[e1],
        })

    return in_maps, {"n_pad": n_pad, "tok_lists": tok_lists}


# --------------------------------------------------------------------------
# device module
# --------------------------------------------------------------------------

def build_module(n_pad, debug=(), stage=99):
    nc = bacc.Bacc("TRN2", target_bir_lowering=False, debug=False,
                   enable_asserts=True, num_devices=NCORES)

    def inp(name, shape, dt=CDT):
        return nc.dram_tensor(name, shape, dt, kind="ExternalInput").ap()

    io = dict(
        x_blk=inp("x_blk", [TPC, D], F32),
        attn_wT=inp("attn_wT", [D, 3 * D]),
        attn_o_wT=inp("attn_o_wT", [D, D]),
        mkeys=inp("mkeys", [D, TE]),
        upT=inp("upT", [D, 2 * DS]),
        downT=inp("downT", [DS, D]),
        w_shared_bc=inp("w_shared_bc", [BLK, D], F32),
        cos_t=inp("cos_t", [BLK, 4 * TPC]),
        sin_t=inp("sin_t", [BLK, 4 * TPC]),
        mask_add=inp("mask_add", [BLK, BLK], F32),
        perm_mats=inp("perm_mats", [BLK, 4 * BLK]),
        negflag0=inp("negflag0", [1, NSLOT0 * BLK]),
        negflag1=inp("negflag1", [1, NSLOT1 * BLK]),
        base_logit=inp("base_logit", [TPC, TOPK], F32),
        router_sel=inp("router_sel", [TPC, TOPK * TE], F32),
        w1=[inp("w1a", [D, ED]), inp("w1b", [D, ED])],
        w2=[inp("w2a", [D, ED]), inp("w2b", [D, ED])],
        w3T=[inp("w3Ta", [ED, D]), inp("w3Tb", [ED, D])],
        idx_e=[inp("idx_a", [n_pad, 1], I32), inp("idx_b", [n_pad, 1], I32)],
        sel_e=[inp("sel_a", [n_pad, TOPK], F32), inp("sel_b", [n_pad, TOPK], F32)],
        out_block=nc.dram_tensor("out_block", [TPC, D], F32,
                                 kind="ExternalOutput").ap(),
        yexp=nc.dram_tensor("yexp", [2 * D, n_pad], F32,
                            kind="ExternalOutput").ap(),
    )
    io["dbg"] = {name: nc.dram_tensor(name, shape, F32, kind="ExternalOutput").ap()
                 for name, shape in debug}

    with tile.TileContext(nc) as tc:
        _body(tc, n_pad, io, stage)
    nc.compile()
    return nc


def _body(tc, n_pad, io, stage=99):
    from contextlib import ExitStack
    nc = tc.nc
    ex = ExitStack()
    rg = [list(range(NCORES))]
    dbg = io["dbg"]

    sb = ex.enter_context(tc.tile_pool(name="sb", bufs=2))
    sbw = ex.enter_context(tc.tile_pool(name="sbw", bufs=4))
    sbk = ex.enter_context(tc.tile_pool(name="sbk", bufs=1))
    psum = ex.enter_context(tc.tile_pool(name="psum", bufs=1, space="PSUM"))
    dram = ex.enter_context(tc.tile_pool(name="dram", bufs=1, space="DRAM"))

    def mmps(n=512):
        return psum.tile([BLK, n], F32, tag="mmps", bufs=3, name="mmps")

    def trps(dt=CDT):
        return psum.tile([BLK, BLK], dt, tag="trps", bufs=2, name="trps")

    def avps():
        return psum.tile([BLK, HD], F32, tag="avps", bufs=2, name="avps")

    identity = sbk.tile([BLK, BLK], CDT, name="identity")
    make_identity(nc, identity[:])
    identity_f = sbk.tile([BLK, BLK], F32, name="identity_f")
    make_identity(nc, identity_f[:])
    ones_row = sbk.tile([1, BLK], CDT, name="ones_row")
    nc.gpsimd.memset(ones_row[:], 1.0)
    ones_f = sbk.tile([1, BLK], F32, name="ones_f")
    nc.gpsimd.memset(ones_f[:], 1.0)
    # const APs used by scalar.activation's implicit bias conversion
    zero_c = sbk.tile([BLK, 1], F32, name="zero_c")
    nc.gpsimd.memset(zero_c[:], 0.0)
    eps_c = sbk.tile([BLK, 1], F32, name="eps_c")
    nc.gpsimd.memset(eps_c[:], EPS)
    nc.const_aps.aps[(F32, 0.0)] = zero_c[:]
    nc.const_aps.aps[(F32, EPS)] = eps_c[:]

    def transpose_to(dst_ap, src_ap):
        pp = trps()
        nc.tensor.transpose(pp[:], src_ap, identity[:])
        nc.vector.tensor_copy(dst_ap, pp[:])

    def rmsnorm_tile(xt_aps, d, out_tiles):
        """xt_aps: list of fp32 APs covering one row-block's d columns.
        out_tiles: list of (dst_ap) same shapes, CDT. Returns rstd."""
        ssums = []
        for a in xt_aps:
            sq = sb.tile([BLK, a.shape[1]], CDT, tag="rms_sq", bufs=2, name="rms_sq")
            ss = sb.tile([BLK, 1], F32, tag="rms_ss", bufs=4, name="rms_ss")
            nc.scalar.activation(sq[:], a, AF.Square, accum_out=ss[:])
            ssums.append(ss)
        tot = ssums[0]
        for s2 in ssums[1:]:
            nc.vector.tensor_tensor(tot[:], tot[:], s2[:], op=OP.add)
        std = sb.tile([BLK, 1], F32, tag="rms_std", bufs=2)
        nc.scalar.activation(std[:], tot[:], AF.Sqrt, scale=1.0 / d, bias=EPS)
        rstd = sb.tile([BLK, 1], F32, tag="rms_rstd", bufs=2)
        nc.vector.reciprocal(rstd[:], std[:])
        for a, o in zip(xt_aps, out_tiles):
            nc.vector.tensor_scalar_mul(o, a, rstd[:])
        return rstd

    # ================= x load + rmsnorm + transpose =================
    x_in = []
    for i in range(2):
        xt = sb.tile([BLK, D], F32, tag="xin4k", bufs=2, name=f"x_in{i}")
        nc.sync.dma_start(xt[:], io["x_blk"][i * BLK:(i + 1) * BLK, :])
        x_in.append(xt)

    xnT = sb.tile([BLK, 8 * TPC], CDT, tag="big", bufs=3, name="xnT")  # free=(dchunk, tok)
    for i in range(2):
        xn = sb.tile([BLK, D], CDT, tag="xn", bufs=2)
        rmsnorm_tile([x_in[i][:]], D, [xn[:]])
        for j in range(8):
            transpose_to(xnT[:, j * TPC + i * BLK: j * TPC + (i + 1) * BLK],
                         xn[:, j * BLK:(j + 1) * BLK])

    # ================= qkv (d-major) =================
    q_sb = sb.tile([BLK, 8 * TPC], CDT, tag="big", bufs=3, name="q_sb")
    k_sb = sb.tile([BLK, 8 * TPC], CDT, tag="big", bufs=3, name="k_sb")
    v_tok = sbk.tile([BLK, 2 * D], CDT, name="v_tok")    # free=(tblk, d)
    for m in range(24):
        ps = mmps(TPC)
        for j in range(8):
            wt = sbw.tile([BLK, BLK], CDT, tag="qkv_w", bufs=6)
            nc.sync.dma_start(wt[:], io["attn_wT"][j * BLK:(j + 1) * BLK,
                                                   m * BLK:(m + 1) * BLK])
            nc.tensor.matmul(ps[:], wt[:], xnT[:, j * TPC:(j + 1) * TPC],
                             start=(j == 0), stop=(j == 7))
        mm = m % 8
        if m < 8:
            nc.vector.tensor_copy(q_sb[:, mm * TPC:(mm + 1) * TPC], ps[:])
        elif m < 16:
            nc.vector.tensor_copy(k_sb[:, mm * TPC:(mm + 1) * TPC], ps[:])
        else:
            vtmp = sb.tile([BLK, TPC], CDT, tag="v_tmp", bufs=2)
            nc.vector.tensor_copy(vtmp[:], ps[:])
            for i in range(2):
                transpose_to(v_tok[:, i * D + mm * BLK: i * D + (mm + 1) * BLK],
                             vtmp[:, i * BLK:(i + 1) * BLK])

    # ================= rope =================
    HALF = 4 * TPC
    cos_sb = sbk.tile([BLK, HALF], CDT, name="cos_sb")
    sin_sb = sbk.tile([BLK, HALF], CDT, name="sin_sb")
    nc.sync.dma_start(cos_sb[:], io["cos_t"][:])
    nc.sync.dma_start(sin_sb[:], io["sin_t"][:])
    for src in (q_sb, k_sb):
        x1c = sb.tile([BLK, HALF], CDT, tag="rope_t", bufs=6)
        x2s = sb.tile([BLK, HALF], CDT, tag="rope_t", bufs=6)
        x2c = sb.tile([BLK, HALF], CDT, tag="rope_t", bufs=6)
        x1s = sb.tile([BLK, HALF], CDT, tag="rope_t", bufs=6)
        nc.vector.tensor_tensor(x1c[:], src[:, :HALF], cos_sb[:], op=OP.mult)
        nc.vector.tensor_tensor(x2s[:], src[:, HALF:], sin_sb[:], op=OP.mult)
        nc.vector.tensor_tensor(x2c[:], src[:, HALF:], cos_sb[:], op=OP.mult)
        nc.vector.tensor_tensor(x1s[:], src[:, :HALF], sin_sb[:], op=OP.mult)
        nc.vector.tensor_tensor(src[:, :HALF], x1c[:], x2s[:], op=OP.add)
        nc.vector.tensor_tensor(src[:, HALF:], x2c[:], x1s[:], op=OP.subtract)

    # permute q,k from rope-split layout to natural head layout
    # (matmul operands must start at partition 0/32/64, so the 4x32 split
    #  packing cannot be addressed directly; natural layout also gives K=64
    #  single-matmul scores)
    pm_sb = sbk.tile([BLK, 4 * BLK], CDT, name="pm_sb")
    nc.sync.dma_start(pm_sb[:], io["perm_mats"][:])
    q_nat = sb.tile([BLK, 8 * TPC], CDT, tag="big", bufs=3, name="q_nat")
    k_nat = sb.tile([BLK, 8 * TPC], CDT, tag="big", bufs=3, name="k_nat")
    for src, dst in ((q_sb, q_nat), (k_sb, k_nat)):
        for jn in range(8):
            par = jn % 2
            js1, js2 = jn // 2, 4 + jn // 2
            ps = mmps(TPC)
            nc.tensor.matmul(ps[:], pm_sb[:, (2 * par) * BLK:(2 * par + 1) * BLK],
                             src[:, js1 * TPC:(js1 + 1) * TPC],
                             start=True, stop=False)
            nc.tensor.matmul(ps[:], pm_sb[:, (2 * par + 1) * BLK:(2 * par + 2) * BLK],
                             src[:, js2 * TPC:(js2 + 1) * TPC],
                             start=False, stop=True)
            nc.vector.tensor_copy(dst[:, jn * TPC:(jn + 1) * TPC], ps[:])

    if stage < 2:
        ex.close()
        return
    # ================= K/V all-gather =================
    k_bounce = dram.tile([2 * 8 * BLK, BLK], CDT, name="k_bounce")
    v_bounce = dram.tile([2 * BLK, D], CDT, name="v_bounce")
    kv_k = dram.tile([NCORES * 2 * 8 * BLK, BLK], CDT, addr_space="Shared",
                     name="kv_k")
    kv_v = dram.tile([NCORES * 2 * BLK, D], CDT, addr_space="Shared",
                     name="kv_v")
    for i in range(2):
        for j in range(8):
            nc.sync.dma_start(k_bounce[(i * 8 + j) * BLK:(i * 8 + j + 1) * BLK, :],
                              k_nat[:, j * TPC + i * BLK: j * TPC + (i + 1) * BLK])
        nc.sync.dma_start(v_bounce[i * BLK:(i + 1) * BLK, :],
                          v_tok[:, i * D:(i + 1) * D])
    nc.gpsimd.collective_compute("AllGather", OP.bypass, replica_groups=rg,
                                 ins=[k_bounce[:]], outs=[kv_k[:]])
    nc.gpsimd.collective_compute("AllGather", OP.bypass, replica_groups=rg,
                                 ins=[v_bounce[:]], outs=[kv_v[:]])

    # gathered K: free=(jj, s, tok); gathered V: free=(s, d)
    k_all = sbk.tile([BLK, 8 * NBLK * BLK], CDT, name="k_all")
    for s in range(NBLK):
        o, i = _owner(s)
        for j in range(8):
            nc.sync.dma_start(
                k_all[:, (j * NBLK + s) * BLK:(j * NBLK + s + 1) * BLK],
                kv_k[(o * 16 + i * 8 + j) * BLK:(o * 16 + i * 8 + j + 1) * BLK, :])

    if stage < 3:
        ex.close()
        return
    # ================= attention =================
    mask_sb = sbk.tile([BLK, BLK], F32, name="mask_sb")
    nc.sync.dma_start(mask_sb[:], io["mask_add"][:])
    nf_sb = [sbk.tile([1, NSLOT0 * BLK], CDT, name="nf0"),
             sbk.tile([1, NSLOT1 * BLK], CDT, name="nf1")]
    nc.sync.dma_start(nf_sb[0][:], io["negflag0"][:])
    nc.sync.dma_start(nf_sb[1][:], io["negflag1"][:])

    oT_all = [sbk.tile([HD, H * BLK], CDT, name=f"oT_all{i}") for i in range(2)]

    for i in range(2):
        nslots = NSLOT0 if i == 0 else NSLOT1
        ngroups = (nslots + 1 + 3) // 4
        for h in range(H):
            hp = (h % 2) * HD                      # partition base 0 or 64
            jn = h // 2
            qh = q_nat[hp:hp + HD, jn * TPC + i * BLK: jn * TPC + (i + 1) * BLK]
            av = avps()
            dsums = []
            for g in range(ngroups):
                s0 = 4 * g
                nsl = min(4, nslots - s0)          # full slots in this group
                has_diag = (g == ngroups - 1)
                ncols = nsl * BLK + (BLK if has_diag else 0)
                ps = mmps()
                # full-chunk scores (K=64), slots s0..s0+nsl-1
                if nsl > 0:
                    rk = k_all[hp:hp + HD,
                               (jn * NBLK + s0) * BLK:(jn * NBLK + s0 + nsl) * BLK]
                    nc.tensor.matmul(ps[:, :nsl * BLK], qh, rk,
                                     start=True, stop=False)
                    nc.tensor.matmul(ps[:, :nsl * BLK], ones_row[:],
                                     nf_sb[i][:, s0 * BLK:(s0 + nsl) * BLK],
                                     start=False, stop=True)
                if has_diag:
                    dk = k_nat[hp:hp + HD,
                               jn * TPC + i * BLK: jn * TPC + (i + 1) * BLK]
                    dc0 = nsl * BLK
                    nc.tensor.matmul(ps[:, dc0:dc0 + BLK], qh, dk,
                                     start=True, stop=True)
                    nc.vector.tensor_tensor(ps[:, dc0:dc0 + BLK],
                                            ps[:, dc0:dc0 + BLK], mask_sb[:],
                                            op=OP.add)
                att = sb.tile([BLK, 512], CDT, tag="att", bufs=3)
                dsum = sb.tile([BLK, 1], F32, tag="dsum", bufs=8)
                nc.scalar.activation(att[:, :ncols], ps[:, :ncols], AF.Exp,
                                     scale=0.125, accum_out=dsum[:])
                dsums.append(dsum)
                # transpose + AV accumulation
                for cc in range(ncols // BLK):
                    attT = sb.tile([BLK, BLK], CDT, tag="attT", bufs=3)
                    pp = trps()
                    nc.tensor.transpose(pp[:], att[:, cc * BLK:(cc + 1) * BLK],
                                        identity[:])
                    nc.vector.tensor_copy(attT[:], pp[:])
                    if has_diag and cc == nsl:
                        vc = v_tok[:, i * D + h * HD: i * D + (h + 1) * HD]
                    else:
                        s = s0 + cc
                        o_s, i_s = _owner(s)
                        vt = sbw.tile([BLK, HD], CDT, tag="v_c", bufs=8, name="v_c")
                        nc.sync.dma_start(
                            vt[:], kv_v[(o_s * 2 + i_s) * BLK:(o_s * 2 + i_s + 1) * BLK,
                                        h * HD:(h + 1) * HD])
                        vc = vt[:]
                    nc.tensor.matmul(av[:], attT[:], vc,
                                     start=(g == 0 and cc == 0),
                                     stop=(g == ngroups - 1 and
                                           cc == ncols // BLK - 1))
            tot = dsums[0]
            for d2 in dsums[1:]:
                nc.vector.tensor_tensor(tot[:], tot[:], d2[:], op=OP.add)
            recip = sb.tile([BLK, 1], F32, tag="recip", bufs=4)
            nc.vector.reciprocal(recip[:], tot[:])
            o_sb = sb.tile([BLK, HD], CDT, tag="o_sb", bufs=3)
            nc.scalar.mul(o_sb[:], av[:], recip[:])
            # transpose to [64, 128] for out-proj lhsT
            pp = trps()
            nc.tensor.transpose(pp[:HD, :], o_sb[:], identity[:])
            nc.vector.tensor_copy(
                oT_all[i][:, h * BLK:(h + 1) * BLK], pp[:HD, :])

    if stage < 4:
        ex.close()
        return
    # ================= out-proj + residual =================
    xfi = [sbk.tile([BLK, D], F32, name=f"xfi{i}") for i in range(2)]
    for dc in range(2):
        ps_i = [mmps(), mmps()]
        for h in range(H):
            wo = sbw.tile([HD, 512], CDT, tag="wo", bufs=4)
            nc.sync.dma_start(wo[:], io["attn_o_wT"][h * HD:(h + 1) * HD,
                                                     dc * 512:(dc + 1) * 512])
            for i in range(2):
                nc.tensor.matmul(ps_i[i][:],
                                 oT_all[i][:, h * BLK:(h + 1) * BLK],
                                 wo[:], start=(h == 0), stop=(h == H - 1))
        for i in range(2):
            nc.vector.tensor_tensor(xfi[i][:, dc * 512:(dc + 1) * 512], ps_i[i][:],
                                    x_in[i][:, dc * 512:(dc + 1) * 512], op=OP.add)

    if stage < 32:
        ex.close()
        return
    if "xfi_dbg" in dbg:
        for i in range(2):
            nc.sync.dma_start(dbg["xfi_dbg"][i * BLK:(i + 1) * BLK, :], xfi[i][:])

    # ================= rmsnorm2 + xfT =================
    xf = [sbk.tile([BLK, D], CDT, name=f"xf{i}") for i in range(2)]
    xfT = sbk.tile([BLK, 8 * TPC], CDT, name="xfT")
    for i in range(2):
        rmsnorm_tile([xfi[i][:]], D, [xf[i][:]])
        for j in range(8):
            transpose_to(xfT[:, j * TPC + i * BLK: j * TPC + (i + 1) * BLK],
                         xf[i][:, j * BLK:(j + 1) * BLK])

    if stage < 33:
        ex.close()
        return
    # ================= router =================
    mk_sb = sbk.tile([BLK, 8 * TE], CDT, name="mk_sb")
    for j in range(8):
        nc.sync.dma_start(mk_sb[:, j * TE:(j + 1) * TE],
                          io["mkeys"][j * BLK:(j + 1) * BLK, :])
    rsel_sb = sbk.tile([BLK, 2 * TOPK * TE], F32, name="rsel_sb")
    for i in range(2):
        nc.sync.dma_start(rsel_sb[:, i * TOPK * TE:(i + 1) * TOPK * TE],
                          io["router_sel"][i * BLK:(i + 1) * BLK, :])
    base_sb = sbk.tile([BLK, 2 * TOPK], F32, name="base_sb")
    for i in range(2):
        nc.sync.dma_start(base_sb[:, i * TOPK:(i + 1) * TOPK],
                          io["base_logit"][i * BLK:(i + 1) * BLK, :])

    scores_sb = sbk.tile([BLK, 2 * TOPK], F32, name="scores_sb")
    for i in range(2):
        vals = psum.tile([BLK, TE], F32, tag="trps", bufs=2, name="vals")
        for j in range(8):
            nc.tensor.matmul(vals[:],
                             xfT[:, j * TPC + i * BLK: j * TPC + (i + 1) * BLK],
                             mk_sb[:, j * TE:(j + 1) * TE],
                             start=(j == 0), stop=(j == 7))
        lg = sb.tile([BLK, TOPK], F32, tag="lg", bufs=2)
        for kk in range(TOPK):
            junk = sb.tile([BLK, TE], F32, tag="rj", bufs=2)
            tvk = sb.tile([BLK, 1], F32, tag="tvk", bufs=2)
            nc.vector.tensor_tensor(
                junk[:], vals[:],
                rsel_sb[:, (i * TOPK + kk) * TE:(i * TOPK + kk + 1) * TE],
                op=OP.mult)
            nc.vector.reduce_sum(tvk[:], junk[:], axis=mybir.AxisListType.X)
            nc.vector.tensor_copy(lg[:, kk:kk + 1], tvk[:])
        nc.vector.tensor_tensor(lg[:], lg[:],
                                base_sb[:, i * TOPK:(i + 1) * TOPK], op=OP.add)
        esum = sb.tile([BLK, 1], F32, tag="esum", bufs=2)
        eexp = sb.tile([BLK, TOPK], F32, tag="eexp", bufs=2)
        nc.scalar.activation(eexp[:], lg[:], AF.Exp, accum_out=esum[:])
        erec = sb.tile([BLK, 1], F32, tag="erec", bufs=2)
        nc.vector.reciprocal(erec[:], esum[:])
        nc.vector.tensor_scalar_mul(scores_sb[:, i * TOPK:(i + 1) * TOPK],
                                    eexp[:], erec[:])

    if "scores_dbg" in dbg:
        for i in range(2):
            nc.sync.dma_start(dbg["scores_dbg"][i * BLK:(i + 1) * BLK, :],
                              scores_sb[:, i * TOPK:(i + 1) * TOPK])

    if stage < 5:
        ex.close()
        return
    # ================= x_ffn + scores all-gather =================
    xf_bounce = dram.tile([TPC, D], CDT, name="xf_bounce")
    sc_bounce = dram.tile([TPC, TOPK], F32, name="sc_bounce")
    xfg = dram.tile([NCORES * TPC, D], CDT, addr_space="Shared", name="xfg")
    scg = dram.tile([NCORES * TPC, TOPK], F32, addr_space="Shared", name="scg")
    for i in range(2):
        nc.sync.dma_start(xf_bounce[i * BLK:(i + 1) * BLK, :], xf[i][:])
        nc.sync.dma_start(sc_bounce[i * BLK:(i + 1) * BLK, :],
                          scores_sb[:, i * TOPK:(i + 1) * TOPK])
    nc.gpsimd.collective_compute("AllGather", OP.bypass, replica_groups=rg,
                                 ins=[xf_bounce[:]], outs=[xfg[:]])
    nc.gpsimd.collective_compute("AllGather", OP.bypass, replica_groups=rg,
                                 ins=[sc_bounce[:]], outs=[scg[:]])

    # ================= shared expert (local tokens) =================
    g_all = sbk.tile([BLK, NBLK * TPC], CDT, name="g_all")   # free=(hchunk, tok)
    for r in range(NBLK):                                    # x1 chunk r, x2 chunk r+16
        ps1 = mmps(TPC)
        ps2 = mmps(TPC)
        for j in range(8):
            u1 = sbw.tile([BLK, BLK], CDT, tag="up_w", bufs=6)
            u2 = sbw.tile([BLK, BLK], CDT, tag="up_w", bufs=6)
            nc.sync.dma_start(u1[:], io["upT"][j * BLK:(j + 1) * BLK,
                                               r * BLK:(r + 1) * BLK])
            nc.sync.dma_start(u2[:], io["upT"][j * BLK:(j + 1) * BLK,
                                               DS + r * BLK: DS + (r + 1) * BLK])
            nc.tensor.matmul(ps1[:], u1[:], xfT[:, j * TPC:(j + 1) * TPC],
                             start=(j == 0), stop=(j == 7))
            nc.tensor.matmul(ps2[:], u2[:], xfT[:, j * TPC:(j + 1) * TPC],
                             start=(j == 0), stop=(j == 7))
        s1 = sb.tile([BLK, TPC], CDT, tag="silu1", bufs=2)
        nc.scalar.activation(s1[:], ps1[:], AF.Sigmoid)
        nc.vector.tensor_tensor(s1[:], s1[:], ps1[:], op=OP.mult)
        nc.vector.tensor_tensor(g_all[:, r * TPC:(r + 1) * TPC], s1[:], ps2[:],
                                op=OP.mult)

    ysh_raw = [sb.tile([BLK, D], F32, tag="xin4k", bufs=2, name=f"ysh_raw{i}") for i in range(2)]
    for dc in range(2):
        ps_i = [mmps(), mmps()]
        for hcc in range(NBLK):
            dw = sbw.tile([BLK, 512], CDT, tag="down_w", bufs=4)
            nc.sync.dma_start(dw[:], io["downT"][hcc * BLK:(hcc + 1) * BLK,
                                                 dc * 512:(dc + 1) * 512])
            for i in range(2):
                nc.tensor.matmul(ps_i[i][:],
                                 g_all[:, hcc * TPC + i * BLK: hcc * TPC + (i + 1) * BLK],
                                 dw[:], start=(hcc == 0), stop=(hcc == NBLK - 1))
        for i in range(2):
            nc.vector.tensor_copy(ysh_raw[i][:, dc * 512:(dc + 1) * 512],
                                  ps_i[i][:])

    wsh_sb = sbk.tile([BLK, D], F32, name="wsh_sb")
    nc.sync.dma_start(wsh_sb[:], io["w_shared_bc"][:])
    for i in range(2):
        ysh = sb.tile([BLK, D], F32, tag="ysh", bufs=2)
        rmsnorm_tile([ysh_raw[i][:]], D, [ysh[:]])
        nc.vector.tensor_tensor(ysh[:], ysh[:], wsh_sb[:], op=OP.mult)
        out_sb = sb.tile([BLK, D], F32, tag="out_sb", bufs=2)
        nc.vector.tensor_tensor(out_sb[:], ysh[:], xfi[i][:], op=OP.add)
        nc.sync.dma_start(io["out_block"][i * BLK:(i + 1) * BLK, :], out_sb[:])

    if stage < 6:
        ex.close()
        return
    # ================= experts =================
    nch = n_pad // BLK
    for le in range(2):
        idx_sb = sb.tile([BLK, nch], I32, tag="idx", bufs=2)
        nc.sync.dma_start(idx_sb[:],
                          io["idx_e"][le].rearrange("(c p) o -> p (c o)", p=BLK))
        # gather x rows + scores; build score row
        xeT = sb.tile([BLK, 8 * n_pad], CDT, tag="xeT", bufs=1)
        srow = sb.tile([1, n_pad], F32, tag="srow", bufs=2)
        for ch in range(nch):
            xe = sb.tile([BLK, D], CDT, tag="xe", bufs=3)
            nc.gpsimd.indirect_dma_start(
                out=xe[:], out_offset=None, in_=xfg[:, :],
                in_offset=bass.IndirectOffsetOnAxis(ap=idx_sb[:, ch:ch + 1], axis=0))
            for j in range(8):
                transpose_to(xeT[:, j * n_pad + ch * BLK: j * n_pad + (ch + 1) * BLK],
                             xe[:, j * BLK:(j + 1) * BLK])
            sc2 = sb.tile([BLK, TOPK], F32, tag="sc2", bufs=3)
            nc.gpsimd.indirect_dma_start(
                out=sc2[:], out_offset=None, in_=scg[:, :],
                in_offset=bass.IndirectOffsetOnAxis(ap=idx_sb[:, ch:ch + 1], axis=0))
            selc = sb.tile([BLK, TOPK], F32, tag="selc", bufs=3)
            nc.sync.dma_start(selc[:], io["sel_e"][le][ch * BLK:(ch + 1) * BLK, :])
            junk = sb.tile([BLK, TOPK], F32, tag="sj", bufs=3)
            wsc = sb.tile([BLK, 1], F32, tag="wsc", bufs=3)
            nc.vector.tensor_tensor(junk[:], sc2[:], selc[:], op=OP.mult)
            nc.vector.reduce_sum(wsc[:], junk[:], axis=mybir.AxisListType.X)
            pp = psum.tile([BLK, BLK], F32, tag="trps", bufs=2, name="ppf")
            nc.tensor.transpose(pp[:1, :], wsc[:], identity_f[:])
            nc.scalar.copy(srow[:, ch * BLK:(ch + 1) * BLK], pp[:1, :])

        for nb in range(0, n_pad, 512):
            ncw = min(512, n_pad - nb)
            # score broadcast across partitions via fp32 rank-1 matmul
            pb = mmps()
            nc.tensor.matmul(pb[:, :ncw], ones_f[:], srow[:, nb:nb + ncw],
                             start=True, stop=True)
            sc_bc = sb.tile([BLK, 512], F32, tag="sc_bc", bufs=1)
            nc.vector.tensor_copy(sc_bc[:, :ncw], pb[:, :ncw])

            gts = []
            for hc in range(4):
                p1 = mmps()
                p2 = mmps()
                for j in range(8):
                    a1 = sbw.tile([BLK, BLK], CDT, tag="e_w", bufs=6)
                    a2 = sbw.tile([BLK, BLK], CDT, tag="e_w", bufs=6)
                    nc.sync.dma_start(a1[:], io["w1"][le][j * BLK:(j + 1) * BLK,
                                                          hc * BLK:(hc + 1) * BLK])
                    nc.sync.dma_start(a2[:], io["w2"][le][j * BLK:(j + 1) * BLK,
                                                          hc * BLK:(hc + 1) * BLK])
                    rx = xeT[:, j * n_pad + nb: j * n_pad + nb + ncw]
                    nc.tensor.matmul(p1[:, :ncw], a1[:], rx, start=(j == 0),
                                     stop=(j == 7))
                    nc.tensor.matmul(p2[:, :ncw], a2[:], rx, start=(j == 0),
                                     stop=(j == 7))
                sg = sb.tile([BLK, 512], CDT, tag="esilu", bufs=2)
                nc.scalar.activation(sg[:, :ncw], p1[:, :ncw], AF.Sigmoid)
                nc.vector.tensor_tensor(sg[:, :ncw], sg[:, :ncw], p1[:, :ncw],
                                        op=OP.mult)
                gt = sb.tile([BLK, 512], CDT, tag="egt", bufs=4)
                nc.vector.tensor_tensor(gt[:, :ncw], sg[:, :ncw], p2[:, :ncw],
                                        op=OP.mult)
                gts.append(gt)
            for dc2 in range(8):
                py = mmps()
                for hc in range(4):
                    w3 = sbw.tile([BLK, BLK], CDT, tag="e_w", bufs=6)
                    nc.sync.dma_start(w3[:], io["w3T"][le][hc * BLK:(hc + 1) * BLK,
                                                           dc2 * BLK:(dc2 + 1) * BLK])
                    nc.tensor.matmul(py[:, :ncw], w3[:], gts[hc][:, :ncw],
                                     start=(hc == 0), stop=(hc == 3))
                ysc = sb.tile([BLK, 512], F32, tag="ysc", bufs=2)
                nc.vector.tensor_tensor(ysc[:, :ncw], py[:, :ncw],
                                        sc_bc[:, :ncw], op=OP.mult)
                nc.sync.dma_start(
                    io["yexp"][le * D + dc2 * BLK: le * D + (dc2 + 1) * BLK,
                               nb:nb + ncw], ysc[:, :ncw])

    ex.close()


# --------------------------------------------------------------------------
# entry point
# --------------------------------------------------------------------------

def kernel(**inputs):
    in_maps, meta = build_host_data(**inputs)
    nc = build_module(meta["n_pad"])
    res = run_bass_kernel_spmd(nc, in_maps, core_ids=list(range(NCORES)))
    return assemble(res.results, meta)


def assemble(results, meta):
    full = np.zeros((S, D), np.float32)
    for c in range(NCORES):
        full[_core_tokens(c)] = results[c]["out_block"]
    for c in range(NCORES):
        for le in range(2):
            e = 2 * c + le
            tl = meta["tok_lists"][e]
            if len(tl):
                yT = results[c]["yexp"][le * D:(le + 1) * D, :len(tl)]
                full[tl] += yT.T
    return full.reshape(1, S, D).astype(np.float32)


